# revision 7
# baseline (speedup 1.0000x reference)
"""AttentionAggregator kernel for 8x Trainium2 NeuronCores.

Math (per graph b):  out[b] = sum_{i in b} softmax_i(score_i) * x_i
  score_i = tanh(x_i @ W1 + b1) @ W2 (+ b2, which cancels in softmax)

Strategy:
  - Host: split the sorted-by-graph node array at segment boundaries so core k
    owns graphs [128k, 128(k+1)) and their nodes; pad node counts to a common
    multiple of 2048; batch ids are rebased to [0,128) (masked/pad nodes -> -1).
    X is downcast to bf16 on host (the on-chip pipeline is bf16 anyway).
  - Device (identical SPMD program on 8 cores), per 2048-node block:
      DMA X block (bf16, flat layout, 257-col strides with a ones column)
      per 512-node megatile:
        PE transpose X -> XT (psum, bf16) -> copy to SBUF (DVE)
        hT = W1.T @ XT (2 psum-accumulated matmuls)          [H=128, 512] f32
        tanh(+b1) on ACT -> bf16 SBUF
        per 128-node tile: scores_col = hT_tanh.T @ W2        [128, 1] psum
        exp on ACT -> e columns
        per 128-node tile: onehot = (iota == batch) * e  (one fused DVE op)
        out_psum[128 segs, 257] += onehot.T @ [X | 1]  (psum-accumulated matmul;
                                    col 256 accumulates the softmax denominator)
  - Epilogue: out = out_psum[:, :256] * 1/max(denom, eps); DMA out.
  - Host: concatenate the 8 [128, 256] blocks -> [1024, 256].
"""

import sys

sys.path.insert(0, "/opt/trn_rl_repo")

import numpy as np
import ml_dtypes

import concourse.bass as bass
import concourse.mybir as mybir
import concourse.tile as tile
from concourse import bacc
from concourse.bass_utils import run_bass_kernel_spmd

N_CORES = 8
D = 256
H = 128
B = 1024
B_LOC = B // N_CORES  # 128 graphs per core
P = 128

TILE_N = 128          # nodes per matmul tile (partition dim)
MEG = 512             # nodes per PSUM megatile = 4 tiles
BLK = 2048            # nodes per DMA block = 4 megatiles
TPB = D + 1           # 257 columns per node-tile in SBUF (data + ones col)

BF16 = mybir.dt.bfloat16
F32 = mybir.dt.float32
NPBF16 = ml_dtypes.bfloat16


def build_program(n_pad: int) -> bass.Bass:
    assert n_pad % BLK == 0
    nblk = n_pad // BLK
    n_meg = n_pad // MEG

    nc = bacc.Bacc(None)
    x_in = nc.dram_tensor("x", [n_pad, D], BF16, kind="ExternalInput")
    bl_in = nc.dram_tensor("bl", [n_pad], F32, kind="ExternalInput")
    w1_in = nc.dram_tensor("w1", [D, H], BF16, kind="ExternalInput")
    w2_in = nc.dram_tensor("w2", [H, 1], BF16, kind="ExternalInput")
    b1_in = nc.dram_tensor("b1", [H, 1], F32, kind="ExternalInput")
    iota_in = nc.dram_tensor("iota", [P, B_LOC], F32, kind="ExternalInput")
    ident_in = nc.dram_tensor("ident", [P, P], BF16, kind="ExternalInput")
    out_ext = nc.dram_tensor("out", [B_LOC, D], F32, kind="ExternalOutput")

    with tile.TileContext(nc) as tc:
        with (
            tc.tile_pool(name="const", bufs=1) as cpool,
            tc.tile_pool(name="xp", bufs=3) as xpool,
            tc.tile_pool(name="xt", bufs=2) as xtpool,
            tc.tile_pool(name="ht", bufs=2) as htpool,
            tc.tile_pool(name="oh", bufs=3) as ohpool,
            tc.tile_pool(name="ep", bufs=2) as epool,
            tc.tile_pool(name="fin", bufs=1) as fpool,
            tc.tile_pool(name="psA", bufs=2, space="PSUM") as psA,
            tc.tile_pool(name="psH", bufs=2, space="PSUM") as psH,
            tc.tile_pool(name="psS", bufs=2, space="PSUM") as psS,
            tc.tile_pool(name="psO", bufs=1, space="PSUM") as psO,
        ):
            # ---- constants (loaded once) ----
            w1_sb = cpool.tile([P, 2, H], BF16)
            nc.sync.dma_start(out=w1_sb[:], in_=w1_in[:, :].rearrange("(c p) h -> p c h", p=P))
            w2_sb = cpool.tile([P, 1], BF16)
            nc.sync.dma_start(out=w2_sb[:], in_=w2_in[:, :])
            b1_sb = cpool.tile([P, 1], F32)
            nc.sync.dma_start(out=b1_sb[:], in_=b1_in[:, :])
            iota_sb = cpool.tile([P, B_LOC], F32)
            nc.sync.dma_start(out=iota_sb[:], in_=iota_in[:, :])
            id_sb = cpool.tile([P, P], BF16)
            nc.sync.dma_start(out=id_sb[:], in_=ident_in[:, :])
            batch_sb = cpool.tile([P, nblk, 16], F32)
            nc.sync.dma_start(
                out=batch_sb[:],
                in_=bl_in[:].rearrange("(k p j) -> p k j", p=P, j=16),
            )

            # persistent output accumulator: [segment, D + denom]
            out_ps = psO.tile([P, TPB], F32)

            mm_idx = 0
            total_outmm = n_meg * 4
            for blk in range(nblk):
                x_sb = xpool.tile([P, 16, TPB], BF16)
                nc.sync.dma_start(
                    out=x_sb[:, :, 0:D],
                    in_=x_in[blk * BLK:(blk + 1) * BLK, :].rearrange(
                        "(p j) d -> p j d", p=P
                    ),
                )
                nc.gpsimd.memset(x_sb[:, :, D:TPB], 1.0)

                for mj in range(4):  # megatiles within block
                    # ---- transpose X -> XT (bf16 via PE) ----
                    xt_ps = psA.tile([P, 2, MEG], BF16)
                    for j4 in range(4):
                        j = mj * 4 + j4
                        for c in range(2):
                            nc.tensor.transpose(
                                xt_ps[:, c, j4 * TILE_N:(j4 + 1) * TILE_N],
                                x_sb[:, j, c * P:(c + 1) * P],
                                id_sb[:],
                            )
                    xt_sb = xtpool.tile([P, 2, MEG], BF16)
                    nc.vector.tensor_copy(out=xt_sb[:], in_=xt_ps[:])

                    # ---- hT = W1.T @ XT  [H, MEG] ----
                    ht_ps = psH.tile([P, MEG], F32)
                    nc.tensor.matmul(
                        ht_ps[:], lhsT=w1_sb[:, 0], rhs=xt_sb[:, 0],
                        start=True, stop=False,
                    )
                    nc.tensor.matmul(
                        ht_ps[:], lhsT=w1_sb[:, 1], rhs=xt_sb[:, 1],
                        start=False, stop=True,
                    )
                    ht_sb = htpool.tile([P, MEG], BF16)
                    nc.scalar.activation(
                        ht_sb[:], ht_ps[:],
                        mybir.ActivationFunctionType.Tanh,
                        bias=b1_sb[:, 0:1], scale=1.0,
                    )

                    # ---- scores (column layout) + exp ----
                    sc_ps = psS.tile([P, 4], F32)
                    for j4 in range(4):
                        nc.tensor.matmul(
                            sc_ps[:, j4:j4 + 1],
                            lhsT=ht_sb[:, j4 * TILE_N:(j4 + 1) * TILE_N],
                            rhs=w2_sb[:],
                        )
                    e_sb = epool.tile([P, 4], F32)
                    nc.scalar.activation(
                        e_sb[:], sc_ps[:], mybir.ActivationFunctionType.Exp
                    )

                    # ---- scaled one-hot + segment-sum matmul ----
                    for j4 in range(4):
                        j = mj * 4 + j4
                        oh_sb = ohpool.tile([P, B_LOC], BF16)
                        nc.vector.tensor_scalar(
                            out=oh_sb[:],
                            in0=iota_sb[:],
                            scalar1=batch_sb[:, blk, j:j + 1],
                            scalar2=e_sb[:, j4:j4 + 1],
                            op0=mybir.AluOpType.is_equal,
                            op1=mybir.AluOpType.mult,
                        )
                        nc.tensor.matmul(
                            out_ps[:],
                            lhsT=oh_sb[:],
                            rhs=x_sb[:, j],
                            start=(mm_idx == 0),
                            stop=(mm_idx == total_outmm - 1),
                        )
                        mm_idx += 1

            # ---- epilogue: divide by denominator ----
            denom_sb = fpool.tile([P, 1], F32)
            nc.vector.tensor_scalar_max(denom_sb[:], out_ps[:, D:TPB], 1e-30)
            rd_sb = fpool.tile([P, 1], F32)
            nc.vector.reciprocal(rd_sb[:], denom_sb[:])
            res_sb = fpool.tile([P, D], F32)
            nc.vector.tensor_scalar(
                out=res_sb[:], in0=out_ps[:, 0:D],
                scalar1=rd_sb[:], scalar2=None,
                op0=mybir.AluOpType.mult,
            )
            nc.sync.dma_start(out=out_ext[:, :], in_=res_sb[:])

    nc.compile()
    return nc


def _shard_inputs(node_features, batch, input_mask, W1, b1, W2, b2):
    """Split at graph boundaries: core k owns graphs [128k, 128(k+1))."""
    n = node_features.shape[0]
    bounds = np.searchsorted(batch, np.arange(0, B + 1, B_LOC))  # 9 split points
    n_locs = np.diff(bounds)
    n_pad = int(max(1, -(-int(n_locs.max()) // BLK))) * BLK

    x_bf = node_features.astype(NPBF16)
    batch_f = batch.astype(np.float32)
    mask = np.asarray(input_mask, dtype=bool)

    iota_np = np.broadcast_to(
        np.arange(B_LOC, dtype=np.float32), (P, B_LOC)
    ).copy()
    ident_np = np.eye(P, dtype=NPBF16)
    w1_np = W1.astype(NPBF16)
    w2_np = W2.astype(NPBF16).reshape(H, 1)
    b1_np = b1.astype(np.float32).reshape(H, 1)

    in_maps = []
    for k in range(N_CORES):
        s, e = int(bounds[k]), int(bounds[k + 1])
        nk = e - s
        xk = np.zeros((n_pad, D), dtype=NPBF16)
        xk[:nk] = x_bf[s:e]
        blk = np.full((n_pad,), -1.0, dtype=np.float32)
        blk[:nk] = np.where(mask[s:e], batch_f[s:e] - k * B_LOC, -1.0)
        in_maps.append({
            "x": xk,
            "bl": blk,
            "w1": w1_np,
            "w2": w2_np,
            "b1": b1_np,
            "iota": iota_np,
            "ident": ident_np,
        })
    return in_maps, n_pad


def kernel(**inputs) -> np.ndarray:
    node_features = np.asarray(inputs["node_features"], dtype=np.float32)
    batch = np.asarray(inputs["batch"])
    input_mask = np.asarray(inputs["input_mask"])
    W1 = np.asarray(inputs["W1"], dtype=np.float32)
    b1 = np.asarray(inputs["b1"], dtype=np.float32)
    W2 = np.asarray(inputs["W2"], dtype=np.float32)
    b2 = np.asarray(inputs["b2"], dtype=np.float32)  # cancels in softmax

    in_maps, n_pad = _shard_inputs(
        node_features, batch, input_mask, W1, b1, W2, b2
    )
    nc = build_program(n_pad)
    res = run_bass_kernel_spmd(nc, in_maps, list(range(N_CORES)), **RUN_KWARGS)
    global LAST_RESULTS
    LAST_RESULTS = res
    out = np.concatenate(
        [res.results[k]["out"] for k in range(N_CORES)], axis=0
    ).astype(np.float32)
    return out


LAST_RESULTS = None
RUN_KWARGS: dict = {}


if __name__ == "__main__":
    # tiny smoke test with random data
    rng = np.random.default_rng(0)
    n = 40000
    x = rng.standard_normal((n, D), dtype=np.float32)
    batch = np.sort(rng.integers(0, B, n).astype(np.int32))
    mask = np.ones((n,), dtype=bool)
    W1 = (rng.standard_normal((D, H)) / np.sqrt(D)).astype(np.float32)
    b1 = np.zeros((H,), dtype=np.float32)
    W2 = (rng.standard_normal((H, 1)) / np.sqrt(H)).astype(np.float32)
    b2 = np.zeros((1,), dtype=np.float32)

    out = kernel(node_features=x, batch=batch, input_mask=mask,
                 W1=W1, b1=b1, W2=W2, b2=b2)

    # numpy reference
    h = np.tanh(x @ W1 + b1)
    s = (h @ W2).ravel() + b2
    e = np.exp(s - s.max())
    num = np.zeros((B, D), dtype=np.float64)
    den = np.zeros((B,), dtype=np.float64)
    np.add.at(num, batch, x * e[:, None])
    np.add.at(den, batch, e)
    exp_out = (num / np.maximum(den, 1e-30)[:, None]).astype(np.float32)

    err = np.abs(out - exp_out).max()
    scale = np.abs(exp_out).max()
    print("abs err:", err, "scale:", scale, "rel:", err / scale)


# revision 11
# speedup vs baseline: 1.0693x; 1.0693x over previous
"""AttentionAggregator kernel for 8x Trainium2 NeuronCores.

Math (per graph b):  out[b] = sum_{i in b} softmax_i(score_i) * x_i
  score_i = tanh(x_i @ W1 + b1) @ W2 (+ b2, which cancels in softmax)

Strategy:
  - Host: split the sorted-by-graph node array at segment boundaries so core k
    owns graphs [128k, 128(k+1)) and their nodes; pad node counts to a common
    multiple of 2048; batch ids are rebased to [0,128) (masked/pad nodes -> -1).
    X is downcast to bf16 on host (the on-chip pipeline is bf16 anyway).
  - Device (identical SPMD program on 8 cores), per 2048-node block:
      DMA X block (bf16, flat layout, 257-col strides with a ones column)
      per 512-node megatile:
        PE transpose X -> XT (psum, bf16) -> copy to SBUF (DVE)
        hT = W1.T @ XT (2 psum-accumulated matmuls)          [H=128, 512] f32
        tanh(+b1) on ACT -> bf16 SBUF
        per 128-node tile: scores_col = hT_tanh.T @ W2        [128, 1] psum
        exp on ACT -> e columns
        per 128-node tile: onehot = (iota == batch) * e  (one fused DVE op)
        out_psum[128 segs, 257] += onehot.T @ [X | 1]  (psum-accumulated matmul;
                                    col 256 accumulates the softmax denominator)
  - Epilogue: out = out_psum[:, :256] * 1/max(denom, eps); DMA out.
  - Host: concatenate the 8 [128, 256] blocks -> [1024, 256].
"""

import sys

sys.path.insert(0, "/opt/trn_rl_repo")

import numpy as np
import ml_dtypes

import concourse.bass as bass
import concourse.mybir as mybir
import concourse.tile as tile
from concourse import bacc
from concourse.bass_utils import run_bass_kernel_spmd

N_CORES = 8
D = 256
H = 128
B = 1024
B_LOC = B // N_CORES  # 128 graphs per core
P = 128

TILE_N = 128          # nodes per matmul tile (partition dim)
MEG = 512             # nodes per PSUM megatile = 4 tiles
BLK = 2048            # nodes per DMA block = 4 megatiles
TPB = D + 1           # 257 columns per node-tile in SBUF (data + ones col)

BF16 = mybir.dt.bfloat16
F32 = mybir.dt.float32
NPBF16 = ml_dtypes.bfloat16


def build_program(n_pad: int) -> bass.Bass:
    assert n_pad % BLK == 0
    nblk = n_pad // BLK
    n_meg = n_pad // MEG

    nc = bacc.Bacc(None)
    x_in = nc.dram_tensor("x", [n_pad, D], BF16, kind="ExternalInput")
    bl_in = nc.dram_tensor("bl", [n_pad], F32, kind="ExternalInput")
    w1_in = nc.dram_tensor("w1", [D, H], BF16, kind="ExternalInput")
    w2_in = nc.dram_tensor("w2", [H, 1], BF16, kind="ExternalInput")
    b1_in = nc.dram_tensor("b1", [H, 1], F32, kind="ExternalInput")
    iota_in = nc.dram_tensor("iota", [P, B_LOC], BF16, kind="ExternalInput")
    ident_in = nc.dram_tensor("ident", [P, P], BF16, kind="ExternalInput")
    out_ext = nc.dram_tensor("out", [B_LOC, D], F32, kind="ExternalOutput")

    with tile.TileContext(nc) as tc:
        with (
            tc.tile_pool(name="const", bufs=1) as cpool,
            tc.tile_pool(name="xp", bufs=3) as xpool,
            tc.tile_pool(name="xt", bufs=3) as xtpool,
            tc.tile_pool(name="ht", bufs=3) as htpool,
            tc.tile_pool(name="oh", bufs=4) as ohpool,
            tc.tile_pool(name="ep", bufs=3) as epool,
            tc.tile_pool(name="fin", bufs=1) as fpool,
            tc.tile_pool(name="psA", bufs=3, space="PSUM") as psA,
            tc.tile_pool(name="psH", bufs=2, space="PSUM") as psH,
            tc.tile_pool(name="psS", bufs=2, space="PSUM") as psS,
            tc.tile_pool(name="psO", bufs=1, space="PSUM") as psO,
        ):
            # ---- constants (loaded once) ----
            w1_sb = cpool.tile([P, 2, H], BF16)
            nc.sync.dma_start(out=w1_sb[:], in_=w1_in[:, :].rearrange("(c p) h -> p c h", p=P))
            w2_sb = cpool.tile([P, 1], BF16)
            nc.sync.dma_start(out=w2_sb[:], in_=w2_in[:, :])
            b1_sb = cpool.tile([P, 1], F32)
            nc.sync.dma_start(out=b1_sb[:], in_=b1_in[:, :])
            iota_sb = cpool.tile([P, B_LOC], BF16)
            nc.sync.dma_start(out=iota_sb[:], in_=iota_in[:, :])
            id_sb = cpool.tile([P, P], BF16)
            nc.sync.dma_start(out=id_sb[:], in_=ident_in[:, :])
            batch_sb = cpool.tile([P, nblk, 16], F32)
            nc.sync.dma_start(
                out=batch_sb[:],
                in_=bl_in[:].rearrange("(k p j) -> p k j", p=P, j=16),
            )

            # persistent output accumulator: [segment, D + denom]
            out_ps = psO.tile([P, TPB], F32)

            mm_idx = 0
            total_outmm = n_meg * 4
            for blk in range(nblk):
                x_sb = xpool.tile([P, 16, TPB], BF16)
                nc.sync.dma_start(
                    out=x_sb[:, :, 0:D],
                    in_=x_in[blk * BLK:(blk + 1) * BLK, :].rearrange(
                        "(p j) d -> p j d", p=P
                    ),
                )
                nc.gpsimd.memset(x_sb[:, :, D:TPB], 1.0)

                for mj in range(4):  # megatiles within block
                    # ---- transpose X -> XT (bf16 via PE) ----
                    xt_ps = psA.tile([P, 2, MEG], BF16)
                    for j4 in range(4):
                        j = mj * 4 + j4
                        for c in range(2):
                            nc.tensor.transpose(
                                xt_ps[:, c, j4 * TILE_N:(j4 + 1) * TILE_N],
                                x_sb[:, j, c * P:(c + 1) * P],
                                id_sb[:],
                            )
                    xt_sb = xtpool.tile([P, 2, MEG], BF16)
                    nc.vector.tensor_copy(out=xt_sb[:], in_=xt_ps[:])

                    # ---- hT = W1.T @ XT  [H, MEG] ----
                    ht_ps = psH.tile([P, MEG], F32)
                    nc.tensor.matmul(
                        ht_ps[:], lhsT=w1_sb[:, 0], rhs=xt_sb[:, 0],
                        start=True, stop=False,
                    )
                    nc.tensor.matmul(
                        ht_ps[:], lhsT=w1_sb[:, 1], rhs=xt_sb[:, 1],
                        start=False, stop=True,
                    )
                    ht_sb = htpool.tile([P, MEG], BF16)
                    nc.scalar.activation(
                        ht_sb[:], ht_ps[:],
                        mybir.ActivationFunctionType.Tanh,
                        bias=b1_sb[:, 0:1], scale=1.0,
                    )

                    # ---- scores (column layout) + exp ----
                    sc_ps = psS.tile([P, 4], F32)
                    for j4 in range(4):
                        nc.tensor.matmul(
                            sc_ps[:, j4:j4 + 1],
                            lhsT=ht_sb[:, j4 * TILE_N:(j4 + 1) * TILE_N],
                            rhs=w2_sb[:],
                        )
                    e_sb = epool.tile([P, 4], F32)
                    nc.scalar.activation(
                        e_sb[:], sc_ps[:], mybir.ActivationFunctionType.Exp
                    )

                    # ---- scaled one-hot + segment-sum matmul ----
                    for j4 in range(4):
                        j = mj * 4 + j4
                        oh_sb = ohpool.tile([P, B_LOC], BF16)
                        nc.vector.tensor_scalar(
                            out=oh_sb[:],
                            in0=iota_sb[:],
                            scalar1=batch_sb[:, blk, j:j + 1],
                            scalar2=e_sb[:, j4:j4 + 1],
                            op0=mybir.AluOpType.is_equal,
                            op1=mybir.AluOpType.mult,
                        )
                        nc.tensor.matmul(
                            out_ps[:],
                            lhsT=oh_sb[:],
                            rhs=x_sb[:, j],
                            start=(mm_idx == 0),
                            stop=(mm_idx == total_outmm - 1),
                        )
                        mm_idx += 1

            # ---- epilogue: divide by denominator ----
            denom_sb = fpool.tile([P, 1], F32)
            nc.vector.tensor_scalar_max(denom_sb[:], out_ps[:, D:TPB], 1e-30)
            rd_sb = fpool.tile([P, 1], F32)
            nc.vector.reciprocal(rd_sb[:], denom_sb[:])
            res_sb = fpool.tile([P, D], F32)
            nc.vector.tensor_scalar(
                out=res_sb[:], in0=out_ps[:, 0:D],
                scalar1=rd_sb[:], scalar2=None,
                op0=mybir.AluOpType.mult,
            )
            nc.sync.dma_start(out=out_ext[:, :], in_=res_sb[:])

    nc.compile()
    return nc


def _shard_inputs(node_features, batch, input_mask, W1, b1, W2, b2):
    """Split at graph boundaries: core k owns graphs [128k, 128(k+1))."""
    n = node_features.shape[0]
    bounds = np.searchsorted(batch, np.arange(0, B + 1, B_LOC))  # 9 split points
    n_locs = np.diff(bounds)
    n_pad = int(max(1, -(-int(n_locs.max()) // BLK))) * BLK

    x_bf = node_features.astype(NPBF16)
    batch_f = batch.astype(np.float32)
    mask = np.asarray(input_mask, dtype=bool)

    iota_np = np.broadcast_to(
        np.arange(B_LOC, dtype=np.float32), (P, B_LOC)
    ).astype(NPBF16)
    ident_np = np.eye(P, dtype=NPBF16)
    w1_np = W1.astype(NPBF16)
    w2_np = W2.astype(NPBF16).reshape(H, 1)
    b1_np = b1.astype(np.float32).reshape(H, 1)

    in_maps = []
    for k in range(N_CORES):
        s, e = int(bounds[k]), int(bounds[k + 1])
        nk = e - s
        xk = np.zeros((n_pad, D), dtype=NPBF16)
        xk[:nk] = x_bf[s:e]
        blk = np.full((n_pad,), -1.0, dtype=np.float32)
        blk[:nk] = np.where(mask[s:e], batch_f[s:e] - k * B_LOC, -1.0)
        in_maps.append({
            "x": xk,
            "bl": blk,
            "w1": w1_np,
            "w2": w2_np,
            "b1": b1_np,
            "iota": iota_np,
            "ident": ident_np,
        })
    return in_maps, n_pad


def kernel(**inputs) -> np.ndarray:
    node_features = np.asarray(inputs["node_features"], dtype=np.float32)
    batch = np.asarray(inputs["batch"])
    input_mask = np.asarray(inputs["input_mask"])
    W1 = np.asarray(inputs["W1"], dtype=np.float32)
    b1 = np.asarray(inputs["b1"], dtype=np.float32)
    W2 = np.asarray(inputs["W2"], dtype=np.float32)
    b2 = np.asarray(inputs["b2"], dtype=np.float32)  # cancels in softmax

    in_maps, n_pad = _shard_inputs(
        node_features, batch, input_mask, W1, b1, W2, b2
    )
    nc = build_program(n_pad)
    res = run_bass_kernel_spmd(nc, in_maps, list(range(N_CORES)), **RUN_KWARGS)
    global LAST_RESULTS
    LAST_RESULTS = res
    out = np.concatenate(
        [res.results[k]["out"] for k in range(N_CORES)], axis=0
    ).astype(np.float32)
    return out


LAST_RESULTS = None
RUN_KWARGS: dict = {}


if __name__ == "__main__":
    # tiny smoke test with random data
    rng = np.random.default_rng(0)
    n = 40000
    x = rng.standard_normal((n, D), dtype=np.float32)
    batch = np.sort(rng.integers(0, B, n).astype(np.int32))
    mask = np.ones((n,), dtype=bool)
    W1 = (rng.standard_normal((D, H)) / np.sqrt(D)).astype(np.float32)
    b1 = np.zeros((H,), dtype=np.float32)
    W2 = (rng.standard_normal((H, 1)) / np.sqrt(H)).astype(np.float32)
    b2 = np.zeros((1,), dtype=np.float32)

    out = kernel(node_features=x, batch=batch, input_mask=mask,
                 W1=W1, b1=b1, W2=W2, b2=b2)

    # numpy reference
    h = np.tanh(x @ W1 + b1)
    s = (h @ W2).ravel() + b2
    e = np.exp(s - s.max())
    num = np.zeros((B, D), dtype=np.float64)
    den = np.zeros((B,), dtype=np.float64)
    np.add.at(num, batch, x * e[:, None])
    np.add.at(den, batch, e)
    exp_out = (num / np.maximum(den, 1e-30)[:, None]).astype(np.float32)

    err = np.abs(out - exp_out).max()
    scale = np.abs(exp_out).max()
    print("abs err:", err, "scale:", scale, "rel:", err / scale)
